# revision 1
# baseline (speedup 1.0000x reference)
"""Trainium2 Bass kernel for the CGC multi-task MoE routing module.

Math: the reference computes, per task t:
    expert outputs  E[t,e] = x @ W[t,e] + b[t,e]          (ES specific + EC common)
    gate logits     L[t]   = concat_e(E[t,e]) @ Wg[t] + bg[t]
    weights         p      = softmax(L[t])
    feature         F[t]   = sum_e p_e * E[t,e]
    out[t]          = F[t] @ Wt[t] + bt[t]                # scalar per sample

Both L[t] and the per-expert scalars s[t,e] = E[t,e] @ Wt[t] are linear in x,
so everything folds into one skinny matmul z = x @ A + d with
A: [I, 24] (per task: 6 logit cols + 6 scalar cols), followed by a per-sample
6-way softmax-weighted average:
    out[t,b] = sum_e exp(L_e) * s_e / sum_e exp(L_e)      (bt folded into s).

v4: x and A are bf16 (tolerance 2e-2; bf16 dot error ~2e-3), halving HBM
traffic.  x loads are all issued up-front, alternating between the two
HWDGE rings (sync + scalar); A/d ride gpsimd SWDGE.  The final band is
split into 128-sample pieces so the post-stream serial chain
(matmul->bias->transpose->exp->reduce->divide->store) runs on tiny
free dims.  The softmax-average uses a fused elementwise divide.
"""

import os

import numpy as np

B, I, H = 65536, 512, 128
T, ES, EC = 2, 2, 4
ETOT = ES + EC

N_CORES = 8
BS = B // N_CORES  # samples per core
M = 32  # folded output channels, padded 24 -> 32 for the 32x32 transpose
GW = 512  # samples per band (one PSUM bank per band)
QW = 4 * GW  # samples per quad (4 bands stacked on the 128 partitions)
NQ = BS // QW
NCHUNK = I // 128
NB = GW // 32  # 32-sample blocks per band

# x loads (samples): 1MB loads for bands 0..13, then a taper so the last
# pieces complete (and can be post-processed) with minimal serial latency
LOADS = [
    (0, 512),
    (512, 512),
    (1024, 1024),
    (2048, 1024),
    (3072, 1024),
    (4096, 1024),
    (5120, 1024),
    (6144, 1024),
    (7168, 512),
    (7680, 512),
]
assert sum(n for _, n in LOADS) == BS


def _fold(inputs):
    """Fold all weights into A [128, NCHUNK, M] (bf16) and bias d [M,1] (f32).

    Channel layout per task t (base 12*t): 0:6 gate logits, 6:12 per-expert
    scalars (bt folded in, valid since softmax weights sum to 1).
    A is packed so that partition p, chunk c holds row c*128+p of the
    [I, M] matrix (matching the xT chunk view).
    """
    import ml_dtypes

    w64 = lambda k: np.asarray(inputs[k], np.float64)
    Wc, bc, Ws, bs = w64("Wc"), w64("bc"), w64("Ws"), w64("bs")
    Wg, bg, Wt, bt = w64("Wg"), w64("bg"), w64("Wt"), w64("bt")

    A = np.zeros((I, M))
    d = np.zeros(M)
    for t in range(T):
        W_all = np.concatenate(
            [Ws[t, e] for e in range(ES)] + [Wc[e] for e in range(EC)], axis=1
        )  # [I, ETOT*H]
        b_all = np.concatenate(
            [bs[t, e] for e in range(ES)] + [bc[e] for e in range(EC)]
        )  # [ETOT*H]
        A[:, 12 * t : 12 * t + 6] = W_all @ Wg[t]
        d[12 * t : 12 * t + 6] = b_all @ Wg[t] + bg[t]
        A[:, 12 * t + 6 : 12 * t + 12] = (
            W_all.reshape(I, ETOT, H) * Wt[t, :, 0][None, None, :]
        ).sum(-1)
        d[12 * t + 6 : 12 * t + 12] = (
            b_all.reshape(ETOT, H) * Wt[t, :, 0][None, :]
        ).sum(-1) + bt[t, 0]
    Apack = (
        A.reshape(NCHUNK, 128, M).transpose(1, 0, 2).astype(ml_dtypes.bfloat16)
    )  # [128, NCHUNK, M]
    return np.ascontiguousarray(Apack), d.reshape(M, 1).astype(np.float32)


def _build_program():
    import concourse.bacc as bacc
    import concourse.mybir as mybir
    from concourse.tile import TileContext

    f32 = mybir.dt.float32
    bf16 = mybir.dt.bfloat16

    nc = bacc.Bacc("TRN2", target_bir_lowering=False, debug=False, num_devices=N_CORES)
    # xT: chunk-strided layout — every load is 128x4 runs of 1-2KB, which
    # keeps all 16 SDMA engines at full duty on a single queue (8KB
    # descriptors run at ~50% duty; sub-512B descriptors pay an RMW penalty)
    xT_ext = nc.declare_dram_parameter("xT", [I, BS], bf16, isOutput=False)
    A_ext = nc.declare_dram_parameter("A", [128, NCHUNK, M], bf16, isOutput=False)
    d_ext = nc.declare_dram_parameter("d", [M, 1], f32, isOutput=False)
    # out[q, p, blk, t]: sample s = q*QW + (p//32)*GW + 32*blk + p%32, task t
    out_ext = nc.declare_dram_parameter("out", [NQ, 128, NB, T], f32, isOutput=True)

    xT_view = xT_ext[:, :].rearrange("(c p) b -> p c b", p=128)  # [128, NCHUNK, BS]

    with TileContext(nc) as tc:
        with (
            tc.tile_pool(name="consts", bufs=1) as cpool,
            tc.tile_pool(name="xin", bufs=1) as xpool,
            tc.tile_pool(name="zt", bufs=3) as ztpool,
            tc.tile_pool(name="zq", bufs=3) as zqpool,
            tc.tile_pool(name="epi", bufs=4) as epool,
            tc.tile_pool(name="psum", bufs=2, space="PSUM") as ppool,
        ):
            # A/d lead the scalar HWDGE ring (fast path; SWDGE descriptor
            # generation would land A ~3.5us late and stall the first matmul)
            A_sb = cpool.tile([128, NCHUNK, M], bf16)
            nc.scalar.dma_start(out=A_sb[:], in_=A_ext[:, :, :])
            d_sb = cpool.tile([M, 1], f32)
            nc.scalar.dma_start(out=d_sb[:], in_=d_ext[:, :])

            # PE p-state pre-warm: the PE ramps to max clock only after ~3us
            # of continuous execution.  Fill the idle window before the first
            # x data lands with short dummy matmuls so all real matmuls run
            # at full clock.
            NWARM = 20
            warm = cpool.tile([128, 64], bf16, name="warm")
            nc.gpsimd.memset(warm[:], 0)
            warm_ps = ppool.tile([M, GW], f32, name="warm_ps", tag="ps0")
            for _ in range(NWARM):
                nc.tensor.matmul(
                    warm_ps[:, 0:64], warm[:, 0:32], warm[:, :], start=True, stop=True
                )

            # prefetch the whole shard on the sync HWDGE ring; L1/L3 ride the
            # scalar ring (issued well before any bias-add queues behind them)
            # so two descriptor streams feed the SDMA engines during the ramp
            xs = []
            for k, (s0, n) in enumerate(LOADS):
                xk = xpool.tile([128, NCHUNK, n], bf16, name=f"x_{k}", tag=f"x{k}")
                eng = nc.scalar if k in (1, 3) else nc.sync
                eng.dma_start(out=xk[:], in_=xT_view[:, :, s0 : s0 + n])
                xs.append(xk)

            # band b (0..15) -> (load tile, sample offset within tile)
            def band_src(b):
                if b < 2:
                    return xs[b], 0  # bands 0, 1: 512KB ramp loads
                if b < 14:
                    return xs[2 + (b - 2) // 2], ((b - 2) % 2) * GW
                return xs[b - 6], 0  # bands 14, 15: dedicated 512KB loads

            from collections import deque

            pending = deque()

            def emit(fn):
                pending.append(fn)
                if len(pending) > 1:
                    pending.popleft()()

            ActT = mybir.ActivationFunctionType
            AxX = mybir.AxisListType.X
            AluAdd = mybir.AluOpType.add
            AluDiv = mybir.AluOpType.divide

            def mm_band(ps_ap, src, c_outer_items=None, *, cols):
                pass  # unused helper placeholder

            def epilogue(idx, Z, npart, nblk, res_t, store_fn):
                """softmax-weighted average on Z [npart, nblk*32]; writes
                res_t [npart, nblk, T]; then store_fn() if given."""
                Zb = Z.rearrange("p (blk c) -> p blk c", c=32)
                zt4 = Zb[:, :, 0:24].rearrange("p blk (t c) -> p blk t c", c=12)
                lg = zt4[:, :, :, 0:6]
                sc = zt4[:, :, :, 6:12]
                # view [.., 2, 6]: group 0 = logits, group 1 = scalars
                both = zt4.rearrange("p blk t (g c) -> p blk t g c", c=6)
                sums = epool.tile(
                    [npart, nblk, T, 2], f32, name=f"sums_{idx}", tag="sums"
                )
                rinv = epool.tile(
                    [npart, nblk, T], f32, name=f"rinv_{idx}", tag="rinv"
                )

                nc.scalar.activation(lg, lg, ActT.Exp)
                nc.vector.tensor_mul(sc, sc, lg)  # sc slot = exp * s
                # one reduce yields both softmax denominator and numerator
                nc.vector.tensor_reduce(sums[:], both, axis=AxX, op=AluAdd)
                nc.vector.reciprocal(rinv[:], sums[:, :, :, 0])
                nc.vector.tensor_mul(res_t, sums[:, :, :, 1], rinv[:])
                if store_fn is not None:
                    store_fn()

            # ---- quads 0..2: full 128-partition pipeline ----
            for q in range(3):
                # release the previous quad's epilogue first: its EXP and
                # vector ops are ready now and fill otherwise-idle slots
                # ahead of this quad's PE-gated bias-adds
                if pending:
                    pending.popleft()()
                zT_sb = ztpool.tile([128, GW], f32, name=f"zTsb_{q}", tag="zTsb")
                pss = [
                    ppool.tile([M, GW], f32, name=f"ps_{q}_{j}", tag=f"ps{j}")
                    for j in range(4)
                ]
                # chunk-outer: consecutive matmuls share stationary weights
                for c in range(NCHUNK):
                    for j in range(4):
                        xk, off = band_src(4 * q + j)
                        nc.tensor.matmul(
                            pss[j][:, :],
                            A_sb[:, c, :],
                            xk[:, c, off : off + GW],
                            start=(c == 0),
                            stop=(c == NCHUNK - 1),
                        )
                # PSUM -> SBUF band writes with per-partition bias add,
                # split across scalar and vector (gpsimd can't read PSUM)
                for j in range(4):
                    dst = zT_sb[32 * j : 32 * j + 32, :]
                    if j < 3:
                        nc.scalar.add(dst, pss[j][:, :], d_sb[:])
                    else:
                        nc.vector.tensor_scalar_add(dst, pss[j][:, :], d_sb[:])

                Zq = zqpool.tile([128, GW], f32, name=f"Z_{q}", tag="Z")
                nc.vector.transpose(Zq[:], zT_sb[:])

                def epi_quad(q=q, Zq=Zq):
                    res = epool.tile([128, NB, T], f32, name=f"res_{q}", tag="res")
                    epilogue(
                        f"q{q}",
                        Zq[:],
                        128,
                        NB,
                        res[:],
                        lambda: nc.sync.dma_start(
                            out=out_ext[q, :, :, :], in_=res[:]
                        ),
                    )

                pending.append(epi_quad)

            # ---- quad 3: single group, band-outer so each band's bias-add
            # starts as soon as its accumulation stops ----
            if pending:
                pending.popleft()()
            zT3 = ztpool.tile([128, GW], f32, name="zTsb_3", tag="zTsb")
            ps3 = [
                ppool.tile([M, GW], f32, name=f"ps_3_{j}", tag=f"ps{j}")
                for j in range(4)
            ]
            for j, b in enumerate([12, 13, 14]):
                xk, off = band_src(b)
                for c in range(NCHUNK):
                    nc.tensor.matmul(
                        ps3[j][:, :],
                        A_sb[:, c, :],
                        xk[:, c, off : off + GW],
                        start=(c == 0),
                        stop=(c == NCHUNK - 1),
                    )
                dst = zT3[32 * j : 32 * j + 32, :]
                if j % 2 == 0:
                    nc.scalar.add(dst, ps3[j][:, :], d_sb[:])
                else:
                    nc.vector.tensor_scalar_add(dst, ps3[j][:, :], d_sb[:])
            xk, off = band_src(15)
            for c in range(NCHUNK):
                nc.tensor.matmul(
                    ps3[3][:, :],
                    A_sb[:, c, :],
                    xk[:, c, off : off + GW],
                    start=(c == 0),
                    stop=(c == NCHUNK - 1),
                )
            nc.vector.tensor_scalar_add(zT3[96:128, :], ps3[3][:, :], d_sb[:])

            Zq3 = zqpool.tile([128, GW], f32, name="Z_3", tag="Z")
            nc.vector.transpose(Zq3[:], zT3[:])

            def epi_q3(Zq3=Zq3):
                res = epool.tile([128, NB, T], f32, name="res_3", tag="res")
                epilogue(
                    "q3",
                    Zq3[:],
                    128,
                    NB,
                    res[:],
                    lambda: nc.sync.dma_start(out=out_ext[3, :, :, :], in_=res[:]),
                )

            pending.append(epi_q3)

            while pending:
                pending.popleft()()

    nc.compile()
    return nc


_PROGRAM = None


def _ensure_ntff_hook():
    """Provide antenv.axon_hooks if the image lacks it (NTFF profiling)."""
    try:
        import antenv.axon_hooks  # noqa: F401

        return
    except ImportError:
        pass
    import contextlib
    import ctypes
    import sys
    import types

    import antenv

    mod = types.ModuleType("antenv.axon_hooks")
    holder = {"hook": None}
    mod.set_axon_ntff_profile_hook = lambda h: holder.__setitem__("hook", h)
    mod.get_axon_ntff_profile_hook = lambda: holder["hook"]
    sys.modules["antenv.axon_hooks"] = mod
    antenv.axon_hooks = mod

    so_path = "/opt/axon/libaxon_pjrt.so"
    try:
        lib = ctypes.CDLL(so_path)
    except OSError:
        return
    if not hasattr(lib, "axon_start_nrt_profile"):
        return
    lib.axon_start_nrt_profile.argtypes = [
        ctypes.POINTER(ctypes.c_int64),
        ctypes.c_size_t,
    ]
    lib.axon_start_nrt_profile.restype = ctypes.c_int64
    lib.axon_stop_nrt_profile.argtypes = [ctypes.c_char_p]
    lib.axon_stop_nrt_profile.restype = ctypes.c_int64

    @contextlib.contextmanager
    def _hook(output_dir, device_ids):
        import jax

        jax.devices()
        if device_ids:
            ids = (ctypes.c_int64 * len(device_ids))(*device_ids)
            rc = lib.axon_start_nrt_profile(ids, len(device_ids))
        else:
            rc = lib.axon_start_nrt_profile(None, 0)
        if rc != 0:
            raise RuntimeError(f"axon_start_nrt_profile rc={rc}")
        try:
            yield
        finally:
            n = lib.axon_stop_nrt_profile(str(output_dir).encode())
            print(f"ntff profile: {n} file(s) written to {output_dir}")

    mod.set_axon_ntff_profile_hook(_hook)


def _run(inputs, trace=False):
    global _PROGRAM
    import ml_dtypes

    import concourse.bass_utils as bass_utils

    if trace:
        _ensure_ntff_hook()
        # keep trace artifacts local; no bucket in this sandbox
        bass_utils.upload_artifacts = lambda tmpdir: "local://" + tmpdir

    A, d = _fold(inputs)
    x = np.asarray(inputs["x"], np.float32)
    in_maps = []
    for i in range(N_CORES):
        shard_T = np.ascontiguousarray(
            x[i * BS : (i + 1) * BS].T.astype(ml_dtypes.bfloat16)
        )  # [I, BS] bf16
        in_maps.append({"xT": shard_T, "A": A, "d": d})

    if _PROGRAM is None:
        _PROGRAM = _build_program()

    kres = bass_utils.run_bass_kernel_spmd(
        _PROGRAM, in_maps, core_ids=list(range(N_CORES)), trace=trace
    )

    parts = []
    for i in range(N_CORES):
        o = np.asarray(kres.results[i]["out"])  # [NQ, 128, NB, T]
        # s = q*QW + j*GW + 32*blk + r with p = 32*j + r
        o = o.reshape(NQ, 4, 32, NB, T)  # q, j, r, blk, t
        parts.append(o.transpose(4, 0, 1, 3, 2).reshape(T, BS))
    full = np.concatenate(parts, axis=1)[:, :, None].astype(np.float32)
    return full, kres


def kernel(**inputs):
    out, _ = _run(inputs, trace=bool(int(os.environ.get("KERNEL_TRACE", "0"))))
    return out



# revision 2
# speedup vs baseline: 1.5673x; 1.5673x over previous
"""Trainium2 Bass kernel for the CGC multi-task MoE routing module.

Math: the reference computes, per task t:
    expert outputs  E[t,e] = x @ W[t,e] + b[t,e]          (ES specific + EC common)
    gate logits     L[t]   = concat_e(E[t,e]) @ Wg[t] + bg[t]
    weights         p      = softmax(L[t])
    feature         F[t]   = sum_e p_e * E[t,e]
    out[t]          = F[t] @ Wt[t] + bt[t]                # scalar per sample
Both L[t] and the per-expert scalars s[t,e] = E[t,e] @ Wt[t] are linear in x,
so everything folds into one skinny matmul z = x @ A + d with A: [I, 24]
(padded to 32), followed by a per-sample 6-way softmax-weighted average.

v5 design (per core, BS=8192 samples):
  - x is quantized to fp8 e3m4 on host (rel err ~1.0e-2 < 2e-2 budget) and
    packed HBM-contiguous per 512-sample band: [16, 128, 4, 512] so each
    band's DMA is one 256KB contiguous read (2KB/partition runs).
  - Loads are issued in strict consumption order on the sync HWDGE ring
    (A first) so band k's semaphore fires ~(k+1)*0.7us into the stream and
    the PE never starves (the v4 all-up-front issue starved the PE until
    ~26us because completion order didn't match consumption order).
  - The matmul keeps A (stationary) in bf16 and streams x moving in fp8e3;
    mixed dtypes multiply exactly in the PE's fp22 path.
  - PE column-group tiling (tile_position=(0,32j)) packs the 4 bands of a
    quad into ONE full PSUM bank [128, 512], so the PSUM->SBUF bias-add is
    a single full-partition scalar op per quad instead of 4 quarter ones.
  - Epilogue per quad: DVE 32x32 block transpose, exp on logit lanes,
    fused softmax-weighted average; the last quad's epilogue is split in
    half to shorten the serial tail.
"""

import os

import numpy as np

B, I, H = 65536, 512, 128
T, ES, EC = 2, 2, 4
ETOT = ES + EC

N_CORES = 8
BS = B // N_CORES  # samples per core
M = 32  # folded output channels, padded 24 -> 32
GW = 512  # samples per band (one PSUM bank column span)
NBAND = BS // GW  # 16 bands
NCHUNK = I // 128
NQ = 4  # quads of 4 bands; one PSUM bank per quad
NB = GW // 32  # 32-sample blocks per band
NWARM = 24  # PE p-state warmup matmuls


def _fold(inputs):
    """Fold all weights into A [128, NCHUNK, M] (bf16) and bias d4 [128,1] f32.

    Channel layout per task t (base 12*t): 0:6 gate logits, 6:12 per-expert
    scalars (bt folded in, valid since softmax weights sum to 1).
    A is packed so that partition p, chunk c holds row c*128+p of the
    [I, M] matrix.  d4 is d tiled x4 across partitions to match the
    4-band col-group PSUM layout.
    """
    import ml_dtypes

    w64 = lambda k: np.asarray(inputs[k], np.float64)
    Wc, bc, Ws, bs = w64("Wc"), w64("bc"), w64("Ws"), w64("bs")
    Wg, bg, Wt, bt = w64("Wg"), w64("bg"), w64("Wt"), w64("bt")

    A = np.zeros((I, M))
    d = np.zeros(M)
    for t in range(T):
        W_all = np.concatenate(
            [Ws[t, e] for e in range(ES)] + [Wc[e] for e in range(EC)], axis=1
        )  # [I, ETOT*H]
        b_all = np.concatenate(
            [bs[t, e] for e in range(ES)] + [bc[e] for e in range(EC)]
        )  # [ETOT*H]
        A[:, 12 * t : 12 * t + 6] = W_all @ Wg[t]
        d[12 * t : 12 * t + 6] = b_all @ Wg[t] + bg[t]
        A[:, 12 * t + 6 : 12 * t + 12] = (
            W_all.reshape(I, ETOT, H) * Wt[t, :, 0][None, None, :]
        ).sum(-1)
        d[12 * t + 6 : 12 * t + 12] = (
            b_all.reshape(ETOT, H) * Wt[t, :, 0][None, :]
        ).sum(-1) + bt[t, 0]
    Apack = (
        A.reshape(NCHUNK, 128, M).transpose(1, 0, 2).astype(ml_dtypes.bfloat16)
    )  # [128, NCHUNK, M]
    d4 = np.tile(d.astype(np.float32), 4).reshape(128, 1)
    return np.ascontiguousarray(Apack), d4


def _build_program():
    import concourse.bacc as bacc
    import concourse.mybir as mybir
    from concourse.tile import TileContext

    f32 = mybir.dt.float32
    bf16 = mybir.dt.bfloat16
    f8 = mybir.dt.float8e3

    nc = bacc.Bacc("TRN2", target_bir_lowering=False, debug=False, num_devices=N_CORES)
    # xp[b]: band b as [128, NCHUNK, GW]; fully contiguous per band so each
    # dma_start is one 256KB read with 2KB/partition contiguous runs.
    xp_ext = nc.declare_dram_parameter("xp", [NBAND, 128, NCHUNK, GW], f8, isOutput=False)
    A_ext = nc.declare_dram_parameter("A", [128, NCHUNK, M], bf16, isOutput=False)
    d4_ext = nc.declare_dram_parameter("d4", [128, 1], f32, isOutput=False)
    # out[q, p, blk, t]: sample s = q*4*GW + (p//32)*GW + 32*blk + p%32, task t
    out_ext = nc.declare_dram_parameter("out", [NQ, 128, NB, T], f32, isOutput=True)

    with TileContext(nc) as tc:
        with (
            tc.tile_pool(name="consts", bufs=1) as cpool,
            tc.tile_pool(name="xin", bufs=1) as xpool,
            tc.tile_pool(name="zt", bufs=2) as ztpool,
            tc.tile_pool(name="zq", bufs=2) as zqpool,
            tc.tile_pool(name="epi", bufs=4) as epool,
            tc.tile_pool(name="psum", bufs=3, space="PSUM") as ppool,
        ):
            # A leads the sync ring (needed before the first matmul);
            # d4 rides the scalar ring (needed only at the first bias-add,
            # ~4us later, behind the implicit ACT table load).
            A_sb = cpool.tile([128, NCHUNK, M], bf16)
            nc.sync.dma_start(out=A_sb[:], in_=A_ext[:, :, :])
            d4_sb = cpool.tile([128, 1], f32)
            nc.scalar.dma_start(out=d4_sb[:], in_=d4_ext[:, :])

            # PE p-state pre-warm: fill the preamble-to-first-data window
            # with dummy matmuls so real matmuls run at full clock.
            warm = cpool.tile([128, 128], bf16, name="warm")
            nc.gpsimd.memset(warm[:], 0)
            warm_ps = ppool.tile([M, 128], f32, name="warm_ps", tag="warm")
            for _ in range(NWARM):
                nc.tensor.matmul(
                    warm_ps[:, :], warm[:, 0:M], warm[:, :], start=True, stop=True
                )

            # x bands in strict consumption order on the sync ring
            xs = []
            for b in range(NBAND):
                xb = xpool.tile([128, NCHUNK, GW], f8, name=f"x_{b}", tag=f"x{b}")
                nc.sync.dma_start(out=xb[:], in_=xp_ext[b, :, :, :])
                xs.append(xb)

            ActT = mybir.ActivationFunctionType
            AxX = mybir.AxisListType.X
            AluAdd = mybir.AluOpType.add

            def epilogue(idx, Z, nblk, res_t):
                """softmax-weighted average on Z [128, nblk*32] -> res_t
                [128, nblk, T]."""
                Zb = Z.rearrange("p (blk c) -> p blk c", c=32)
                zt4 = Zb[:, :, 0:24].rearrange("p blk (t c) -> p blk t c", c=12)
                lg = zt4[:, :, :, 0:6]
                sc = zt4[:, :, :, 6:12]
                both = zt4.rearrange("p blk t (g c) -> p blk t g c", c=6)
                sums = epool.tile([128, nblk, T, 2], f32, name=f"sums_{idx}", tag="sums")
                rinv = epool.tile([128, nblk, T], f32, name=f"rinv_{idx}", tag="rinv")
                nc.scalar.activation(lg, lg, ActT.Exp)
                nc.vector.tensor_mul(sc, sc, lg)  # sc slot = exp * s
                nc.vector.tensor_reduce(sums[:], both, axis=AxX, op=AluAdd)
                nc.vector.reciprocal(rinv[:], sums[:, :, :, 0])
                nc.vector.tensor_mul(res_t, sums[:, :, :, 1], rinv[:])

            for q in range(NQ):
                psZ = ppool.tile([128, GW], f32, name=f"psZ_{q}", tag="psZ")
                # band-outer: band j's 4 chunk-matmuls start as soon as its
                # load lands; col group j -> psum partitions 32j:32j+32
                for j in range(4):
                    xb = xs[4 * q + j]
                    for c in range(NCHUNK):
                        nc.tensor.matmul(
                            psZ[32 * j : 32 * j + 32, :],
                            A_sb[:, c, :],
                            xb[:, c, :],
                            start=(c == 0),
                            stop=(c == NCHUNK - 1),
                            tile_position=(0, 32 * j),
                        )
                # single full-partition PSUM->SBUF bias-add
                zT = ztpool.tile([128, GW], f32, name=f"zT_{q}", tag="zT")
                nc.scalar.add(zT[:], psZ[:], d4_sb[:])
                Zq = zqpool.tile([128, GW], f32, name=f"Z_{q}", tag="Z")
                nc.vector.transpose(Zq[:], zT[:])
                if q < NQ - 1:
                    res = epool.tile([128, NB, T], f32, name=f"res_{q}", tag="res")
                    epilogue(q, Zq[:], NB, res[:])
                    nc.sync.dma_start(out=out_ext[q, :, :, :], in_=res[:])
                else:
                    # split the last epilogue to shorten the serial tail
                    for h in range(2):
                        nh = NB // 2
                        res = epool.tile(
                            [128, nh, T], f32, name=f"res_{q}_{h}", tag="res"
                        )
                        epilogue(f"{q}_{h}", Zq[:, h * 256 : (h + 1) * 256], nh, res[:])
                        nc.sync.dma_start(
                            out=out_ext[q, :, h * nh : (h + 1) * nh, :], in_=res[:]
                        )

    nc.compile()
    return nc


_PROGRAM = None


def _ensure_ntff_hook():
    """Provide antenv.axon_hooks if the image lacks it (NTFF profiling)."""
    try:
        import antenv.axon_hooks  # noqa: F401

        return
    except ImportError:
        pass
    import contextlib
    import ctypes
    import sys
    import types

    import antenv

    mod = types.ModuleType("antenv.axon_hooks")
    holder = {"hook": None}
    mod.set_axon_ntff_profile_hook = lambda h: holder.__setitem__("hook", h)
    mod.get_axon_ntff_profile_hook = lambda: holder["hook"]
    sys.modules["antenv.axon_hooks"] = mod
    antenv.axon_hooks = mod

    so_path = "/opt/axon/libaxon_pjrt.so"
    try:
        lib = ctypes.CDLL(so_path)
    except OSError:
        return
    if not hasattr(lib, "axon_start_nrt_profile"):
        return
    lib.axon_start_nrt_profile.argtypes = [
        ctypes.POINTER(ctypes.c_int64),
        ctypes.c_size_t,
    ]
    lib.axon_start_nrt_profile.restype = ctypes.c_int64
    lib.axon_stop_nrt_profile.argtypes = [ctypes.c_char_p]
    lib.axon_stop_nrt_profile.restype = ctypes.c_int64

    @contextlib.contextmanager
    def _hook(output_dir, device_ids):
        import jax

        jax.devices()
        if device_ids:
            ids = (ctypes.c_int64 * len(device_ids))(*device_ids)
            rc = lib.axon_start_nrt_profile(ids, len(device_ids))
        else:
            rc = lib.axon_start_nrt_profile(None, 0)
        if rc != 0:
            raise RuntimeError(f"axon_start_nrt_profile rc={rc}")
        try:
            yield
        finally:
            n = lib.axon_stop_nrt_profile(str(output_dir).encode())
            print(f"ntff profile: {n} file(s) written to {output_dir}")

    mod.set_axon_ntff_profile_hook(_hook)


def _run(inputs, trace=False):
    global _PROGRAM
    import ml_dtypes

    import concourse.bass_utils as bass_utils

    if trace:
        _ensure_ntff_hook()
        bass_utils.upload_artifacts = lambda tmpdir: "local://" + tmpdir

    A, d4 = _fold(inputs)
    x8 = np.asarray(inputs["x"], np.float32).astype(ml_dtypes.float8_e3m4)
    in_maps = []
    for i in range(N_CORES):
        shard = x8[i * BS : (i + 1) * BS]  # [BS, I] fp8
        # xp[b, p, c, s] = x[b*GW + s, c*128 + p]
        xp = np.ascontiguousarray(
            shard.T.reshape(NCHUNK, 128, NBAND, GW).transpose(2, 1, 0, 3)
        )
        in_maps.append({"xp": xp, "A": A, "d4": d4})

    if _PROGRAM is None:
        _PROGRAM = _build_program()

    kres = bass_utils.run_bass_kernel_spmd(
        _PROGRAM, in_maps, core_ids=list(range(N_CORES)), trace=trace
    )

    parts = []
    for i in range(N_CORES):
        o = np.asarray(kres.results[i]["out"])  # [NQ, 128, NB, T]
        # s = q*4*GW + j*GW + 32*blk + r with p = 32*j + r
        o = o.reshape(NQ, 4, 32, NB, T)  # q, j, r, blk, t
        parts.append(o.transpose(4, 0, 1, 3, 2).reshape(T, BS))
    full = np.concatenate(parts, axis=1)[:, :, None].astype(np.float32)
    return full, kres


def kernel(**inputs):
    out, _ = _run(inputs, trace=bool(int(os.environ.get("KERNEL_TRACE", "0"))))
    return out


# revision 5
# speedup vs baseline: 1.5780x; 1.0068x over previous
"""Trainium2 Bass kernel for the CGC multi-task MoE routing module.

Math: everything folds into one skinny matmul z = x @ A + d with A: [I, 24]
(padded to 32), followed by a per-sample 6-way softmax-weighted average
(see _fold).

v6 design (per core, BS=8192 samples):
  - x quantized to fp8 e3m4 on host (rel err ~1.0e-2 < 2e-2 budget), packed
    HBM-contiguous per 512-sample band: [16, 128, 4, 512] so each band DMA
    is one 256KB contiguous read (one 2KB descriptor per partition).
  - Band loads split across BOTH HWDGE rings (even->sync, odd->scalar) in
    consumption order: a single ring's descriptor generation (~650ns/load,
    128 descriptors) paces the stream at ~290 GB/s; two rings restore line
    rate (~380 GB/s).  A leads the sync ring; d4 rides gpsimd SWDGE.
  - Matmul: stationary A bf16, moving x fp8e3 (mixed dtypes multiply
    exactly in the PE fp22 path).  PE column-group tiling
    (tile_position=(0,32j)) packs a quad's 4 bands into ONE PSUM bank
    [128, 512] -> a single full-partition bias-add per quad.
  - Epilogue per quad: DVE 32x32 block transpose, exp on logit lanes,
    softmax-weighted average with reciprocal_approx_fast (1 DVE op, ~51
    ULP - plenty for the softmax denominator in [4, 20]).
  - The last quad is split into two 256-sample column halves with separate
    PSUM tiles and epilogue chains so the post-stream serial tail is
    halved.
"""

import os

import numpy as np

B, I, H = 65536, 512, 128
T, ES, EC = 2, 2, 4
ETOT = ES + EC

N_CORES = 8
BS = B // N_CORES  # samples per core
M = 32  # folded output channels, padded 24 -> 32
GW = 512  # samples per band (one PSUM bank column span)
NBAND = BS // GW  # 16 bands
NCHUNK = I // 128
NQ = 4  # quads of 4 bands; one PSUM bank per quad
NB = GW // 32  # 32-sample blocks per band
NWARM = 24  # PE p-state warmup matmuls


def _fold(inputs):
    """Fold all weights into A [128, NCHUNK, M] (bf16) and bias d4 [128,1] f32.

    Channel layout per task t (base 12*t): 0:6 gate logits, 6:12 per-expert
    scalars (bt folded in, valid since softmax weights sum to 1).
    A is packed so that partition p, chunk c holds row c*128+p of the
    [I, M] matrix.  d4 is d tiled x4 across partitions to match the
    4-band col-group PSUM layout.
    """
    import ml_dtypes

    w64 = lambda k: np.asarray(inputs[k], np.float64)
    Wc, bc, Ws, bs = w64("Wc"), w64("bc"), w64("Ws"), w64("bs")
    Wg, bg, Wt, bt = w64("Wg"), w64("bg"), w64("Wt"), w64("bt")

    A = np.zeros((I, M))
    d = np.zeros(M)
    for t in range(T):
        W_all = np.concatenate(
            [Ws[t, e] for e in range(ES)] + [Wc[e] for e in range(EC)], axis=1
        )  # [I, ETOT*H]
        b_all = np.concatenate(
            [bs[t, e] for e in range(ES)] + [bc[e] for e in range(EC)]
        )  # [ETOT*H]
        A[:, 12 * t : 12 * t + 6] = W_all @ Wg[t]
        d[12 * t : 12 * t + 6] = b_all @ Wg[t] + bg[t]
        A[:, 12 * t + 6 : 12 * t + 12] = (
            W_all.reshape(I, ETOT, H) * Wt[t, :, 0][None, None, :]
        ).sum(-1)
        d[12 * t + 6 : 12 * t + 12] = (
            b_all.reshape(ETOT, H) * Wt[t, :, 0][None, :]
        ).sum(-1) + bt[t, 0]
    Apack = (
        A.reshape(NCHUNK, 128, M).transpose(1, 0, 2).astype(ml_dtypes.bfloat16)
    )  # [128, NCHUNK, M]
    d4 = np.tile(d.astype(np.float32), 4).reshape(128, 1)
    return np.ascontiguousarray(Apack), d4


def _build_program():
    import concourse.bacc as bacc
    import concourse.mybir as mybir
    from concourse.tile import TileContext

    f32 = mybir.dt.float32
    bf16 = mybir.dt.bfloat16
    f8 = mybir.dt.float8e3

    nc = bacc.Bacc("TRN2", target_bir_lowering=False, debug=False, num_devices=N_CORES)
    xp_ext = nc.declare_dram_parameter("xp", [NBAND, 128, NCHUNK, GW], f8, isOutput=False)
    A_ext = nc.declare_dram_parameter("A", [128, NCHUNK, M], bf16, isOutput=False)
    d4_ext = nc.declare_dram_parameter("d4", [128, 1], f32, isOutput=False)
    # out[q, p, blk, t]: sample s = q*4*GW + (p//32)*GW + 32*blk + p%32, task t
    out_ext = nc.declare_dram_parameter("out", [NQ, 128, NB, T], f32, isOutput=True)

    with TileContext(nc) as tc:
        with (
            tc.tile_pool(name="consts", bufs=1) as cpool,
            tc.tile_pool(name="xin", bufs=1) as xpool,
            tc.tile_pool(name="zt", bufs=2) as ztpool,
            tc.tile_pool(name="zq", bufs=2) as zqpool,
            tc.tile_pool(name="epi", bufs=4) as epool,
            tc.tile_pool(name="psum", bufs=3, space="PSUM") as ppool,
        ):
            # d4 via gpsimd SWDGE: frees both HWDGE rings; needed only at the
            # first bias-add (~12us), SWDGE latency is fine.
            d4_sb = cpool.tile([128, 1], f32)
            nc.gpsimd.dma_start(out=d4_sb[:], in_=d4_ext[:, :])
            # A leads the sync ring (needed by the first matmul)
            A_sb = cpool.tile([128, NCHUNK, M], bf16)
            nc.sync.dma_start(out=A_sb[:], in_=A_ext[:, :, :])

            # PE p-state pre-warm: fill the preamble-to-first-data window.
            warm = cpool.tile([128, 128], bf16, name="warm")
            nc.gpsimd.memset(warm[:], 0)
            warm_ps = ppool.tile([M, 128], f32, name="warm_ps", tag="warm", bufs=1)
            for _ in range(NWARM):
                nc.tensor.matmul(
                    warm_ps[:, :], warm[:, 0:M], warm[:, :], start=True, stop=True
                )

            # x bands in consumption order, alternating HWDGE rings
            xs = []
            for b in range(NBAND):
                xb = xpool.tile([128, NCHUNK, GW], f8, name=f"x_{b}", tag=f"x{b}")
                eng = nc.sync if b % 2 == 0 else nc.scalar
                eng.dma_start(out=xb[:], in_=xp_ext[b, :, :, :])
                xs.append(xb)

            ActT = mybir.ActivationFunctionType
            AxX = mybir.AxisListType.X
            AluAdd = mybir.AluOpType.add

            def epilogue(idx, Z, nblk, res_t):
                """softmax-weighted average on Z [128, nblk*32] -> res_t
                [128, nblk, T]."""
                Zb = Z.rearrange("p (blk c) -> p blk c", c=32)
                zt4 = Zb[:, :, 0:24].rearrange("p blk (t c) -> p blk t c", c=12)
                lg = zt4[:, :, :, 0:6]
                sc = zt4[:, :, :, 6:12]
                both = zt4.rearrange("p blk t (g c) -> p blk t g c", c=6)
                sums = epool.tile([128, nblk, T, 2], f32, name=f"sums_{idx}", tag="sums")
                rinv = epool.tile([128, nblk, T], f32, name=f"rinv_{idx}", tag="rinv")
                nc.scalar.activation(lg, lg, ActT.Exp)
                nc.vector.tensor_mul(sc, sc, lg)  # sc slot = exp * s
                nc.vector.tensor_reduce(sums[:], both, axis=AxX, op=AluAdd)
                nc.vector.reciprocal_approx_fast(out=rinv[:], in_=sums[:, :, :, 0])
                nc.vector.tensor_mul(res_t, sums[:, :, :, 1], rinv[:])

            def quad_chain(idx, psrc, ncols, out_slice):
                """bias-add + transpose + epilogue + store for psrc
                [128, ncols]; out_slice is the out_ext destination."""
                nblk = ncols // 32
                zT = ztpool.tile([128, ncols], f32, name=f"zT_{idx}", tag="zT")
                nc.scalar.add(zT[:], psrc, d4_sb[:])
                Zq = zqpool.tile([128, ncols], f32, name=f"Z_{idx}", tag="Z")
                nc.vector.transpose(Zq[:], zT[:])
                res = epool.tile([128, nblk, T], f32, name=f"res_{idx}", tag="res")
                epilogue(idx, Zq[:], nblk, res[:])
                nc.sync.dma_start(out=out_slice, in_=res[:])

            for q in range(NQ - 1):
                psZ = ppool.tile([128, GW], f32, name=f"psZ_{q}", tag="psZ")
                # band-outer: band j's 4 chunk-matmuls start as soon as its
                # load lands; col group j -> psum partitions 32j:32j+32
                for j in range(4):
                    xb = xs[4 * q + j]
                    for c in range(NCHUNK):
                        nc.tensor.matmul(
                            psZ[32 * j : 32 * j + 32, :],
                            A_sb[:, c, :],
                            xb[:, c, :],
                            start=(c == 0),
                            stop=(c == NCHUNK - 1),
                            tile_position=(0, 32 * j),
                        )
                quad_chain(q, psZ[:], GW, out_ext[q, :, :, :])

            # last quad: two 256-sample column halves with separate PSUM
            # tiles and chains, halving the post-stream serial tail
            HW_ = GW // 2
            ps3 = [
                ppool.tile([128, HW_], f32, name=f"ps3_{h}", tag=f"ps3{h}", bufs=1)
                for h in range(2)
            ]
            for j in range(4):
                xb = xs[12 + j]
                for h in range(2):
                    for c in range(NCHUNK):
                        nc.tensor.matmul(
                            ps3[h][32 * j : 32 * j + 32, :],
                            A_sb[:, c, :],
                            xb[:, c, h * HW_ : (h + 1) * HW_],
                            start=(c == 0),
                            stop=(c == NCHUNK - 1),
                            tile_position=(0, 32 * j),
                        )
            nh = NB // 2
            for h in range(2):
                quad_chain(
                    f"3_{h}", ps3[h][:], HW_, out_ext[3, :, h * nh : (h + 1) * nh, :]
                )

    nc.compile()
    return nc


_PROGRAM = None


def _ensure_ntff_hook():
    """Provide antenv.axon_hooks if the image lacks it (NTFF profiling)."""
    try:
        import antenv.axon_hooks  # noqa: F401

        return
    except ImportError:
        pass
    import contextlib
    import ctypes
    import sys
    import types

    import antenv

    mod = types.ModuleType("antenv.axon_hooks")
    holder = {"hook": None}
    mod.set_axon_ntff_profile_hook = lambda h: holder.__setitem__("hook", h)
    mod.get_axon_ntff_profile_hook = lambda: holder["hook"]
    sys.modules["antenv.axon_hooks"] = mod
    antenv.axon_hooks = mod

    so_path = "/opt/axon/libaxon_pjrt.so"
    try:
        lib = ctypes.CDLL(so_path)
    except OSError:
        return
    if not hasattr(lib, "axon_start_nrt_profile"):
        return
    lib.axon_start_nrt_profile.argtypes = [
        ctypes.POINTER(ctypes.c_int64),
        ctypes.c_size_t,
    ]
    lib.axon_start_nrt_profile.restype = ctypes.c_int64
    lib.axon_stop_nrt_profile.argtypes = [ctypes.c_char_p]
    lib.axon_stop_nrt_profile.restype = ctypes.c_int64

    @contextlib.contextmanager
    def _hook(output_dir, device_ids):
        import jax

        jax.devices()
        if device_ids:
            ids = (ctypes.c_int64 * len(device_ids))(*device_ids)
            rc = lib.axon_start_nrt_profile(ids, len(device_ids))
        else:
            rc = lib.axon_start_nrt_profile(None, 0)
        if rc != 0:
            raise RuntimeError(f"axon_start_nrt_profile rc={rc}")
        try:
            yield
        finally:
            n = lib.axon_stop_nrt_profile(str(output_dir).encode())
            print(f"ntff profile: {n} file(s) written to {output_dir}")

    mod.set_axon_ntff_profile_hook(_hook)


def _run(inputs, trace=False):
    global _PROGRAM
    import ml_dtypes

    import concourse.bass_utils as bass_utils

    if trace:
        _ensure_ntff_hook()
        bass_utils.upload_artifacts = lambda tmpdir: "local://" + tmpdir

    A, d4 = _fold(inputs)
    x8 = np.asarray(inputs["x"], np.float32).astype(ml_dtypes.float8_e3m4)
    in_maps = []
    for i in range(N_CORES):
        shard = x8[i * BS : (i + 1) * BS]  # [BS, I] fp8
        # xp[b, p, c, s] = x[b*GW + s, c*128 + p]
        xp = np.ascontiguousarray(
            shard.T.reshape(NCHUNK, 128, NBAND, GW).transpose(2, 1, 0, 3)
        )
        in_maps.append({"xp": xp, "A": A, "d4": d4})

    if _PROGRAM is None:
        _PROGRAM = _build_program()

    kres = bass_utils.run_bass_kernel_spmd(
        _PROGRAM, in_maps, core_ids=list(range(N_CORES)), trace=trace
    )

    parts = []
    for i in range(N_CORES):
        o = np.asarray(kres.results[i]["out"])  # [NQ, 128, NB, T]
        # s = q*4*GW + j*GW + 32*blk + r with p = 32*j + r
        o = o.reshape(NQ, 4, 32, NB, T)  # q, j, r, blk, t
        parts.append(o.transpose(4, 0, 1, 3, 2).reshape(T, BS))
    full = np.concatenate(parts, axis=1)[:, :, None].astype(np.float32)
    return full, kres


def kernel(**inputs):
    out, _ = _run(inputs, trace=bool(int(os.environ.get("KERNEL_TRACE", "0"))))
    return out
